# revision 29
# baseline (speedup 1.0000x reference)
"""Trainium2 Bass kernel for nn_DepthCueRectification_Sp.

Data-parallel over batch: 8 batch elements -> 8 NeuronCores (SPMD).

Fully software-pipelined single-pass design (all GEMMs bf16):
  prologue: xs_k.T = A_k.T @ xb.T   (A_k = U.T diag|S_k| U folded on host)
            pos chains for blocks 0,1 on DVE/Pool
  windows nb=0..7 (one attention row-block each):
    tensor: logits(nb) -> transpose(nb-1) -> y_outT(nb-1) -> MLP1 pass-A slice
    ACT:    softmax exps (no max-sub), entropy ln, psum drains, gelu
    DVE:    pos chain A, entropy fused mul+reduce, routing
    Pool:   pos chain B, attn mix, route-combine (dka)
  tail: heat shift-matmul, MLP2-A, MLP1-B, MLP2-B, MLP1-C, MLP2-C,
        residual + heat scaling + store per 256-col chunk.

Routing simplification: hmap0>=hmap1 <=> entS0>=entS1 where
entS = sum(attn*ln(attn+eps)) (= -entropy), and
heat = 2/(1+exp(-h_temp*max(entS0,entS1))). The CLS row's heat=1 and the
(token -> output row+1) shift are realized with a tiny shift-matrix matmul.
"""

import sys

if "/opt/trn_rl_repo" not in sys.path:
    sys.path.insert(0, "/opt/trn_rl_repo")

import numpy as np
import ml_dtypes

import concourse.bass as bass
import concourse.mybir as mybir
import concourse.tile as tile
from concourse import bacc
from concourse.bass_utils import run_bass_kernel_spmd
from concourse.masks import make_identity

B, N, D, DFF, CLS = 8, 1024, 768, 3072, 1
NP1 = N + CLS          # 1025
NPAD = 1152            # 9 * 128
ND = D // 128          # 6
NB = N // 128          # 8
NF = DFF // 128        # 24
NC2 = 2 * D // 128     # 12
NTB = NPAD // 128      # 9 output token blocks
AF = mybir.ActivationFunctionType
ALU = mybir.AluOpType
dt = mybir.dt

# MLP token-column passes over the padded 1152 columns
PASSES = [(0, 512), (512, 512), (1024, 128)]

_prog_cache = {}


def _build(g, ht, pt):
    scale = float(D) ** -0.5
    omg = 1.0 - g

    nc = bacc.Bacc("TRN2", target_bir_lowering=False, debug=False, num_devices=8)

    xtb_d = nc.declare_dram_parameter("xtb", [D, NPAD], dt.bfloat16, isOutput=False)
    ytb_d = nc.declare_dram_parameter("ytb", [D, NP1], dt.bfloat16, isOutput=False)
    ybn_d = nc.declare_dram_parameter("ybn", [N, D], dt.bfloat16, isOutput=False)
    ap_d = nc.declare_dram_parameter("app", [2, 128, ND * D], dt.bfloat16, isOutput=False)
    cpl_d = nc.declare_dram_parameter("cplp", [NB, 128, 6 * N], dt.bfloat16, isOutput=False)
    pem_d = nc.declare_dram_parameter("pem", [N, 6], dt.float32, isOutput=False)
    w1p_d = nc.declare_dram_parameter("w1p", [NF, 128, NC2 * 128], dt.bfloat16, isOutput=False)
    w2r_d = nc.declare_dram_parameter("w2r", [DFF, D], dt.bfloat16, isOutput=False)
    b1t_d = nc.declare_dram_parameter("b1t", [128, NF], dt.float32, isOutput=False)
    b2b_d = nc.declare_dram_parameter("b2b", [128, D], dt.float32, isOutput=False)
    xna_d = nc.declare_dram_parameter("xna", [NPAD, D], dt.float32, isOutput=False)
    out_d = nc.declare_dram_parameter("out", [NPAD, D], dt.float32, isOutput=True)
    hmbuf = nc.dram_tensor("hmbuf", [NPAD, 1], dt.float32)

    with tile.TileContext(nc) as tc:
        with tc.tile_pool(name="p0", bufs=1) as P0, \
             tc.tile_pool(name="pws", bufs=2) as WS, \
             tc.tile_pool(name="pwA", bufs=1) as WSA, \
             tc.tile_pool(name="pwB", bufs=1) as WSB, \
             tc.tile_pool(name="pwD", bufs=3) as WSD, \
             tc.tile_pool(name="pcpl", bufs=2) as PC, \
             tc.tile_pool(name="ppet", bufs=3) as PPET, \
             tc.tile_pool(name="psml", bufs=2) as SM, \
             tc.tile_pool(name="pact", bufs=1) as PACT, \
             tc.tile_pool(name="psA", bufs=3, space="PSUM") as PA, \
             tc.tile_pool(name="psT", bufs=1, space="PSUM") as PT, \
             tc.tile_pool(name="psH", bufs=2, space="PSUM") as PH, \
             tc.tile_pool(name="psO", bufs=2, space="PSUM") as PO:

            # ---------------- persistent tiles ----------------
            xtb = [P0.tile([128, NPAD], dt.bfloat16, tag=f"xtb{d}", name=f"xtb{d}") for d in range(ND)]
            ytb = [P0.tile([128, NP1], dt.bfloat16, tag=f"ytb{d}", name=f"ytb{d}") for d in range(ND)]
            ybn = [P0.tile([128, D], dt.bfloat16, tag=f"ybn{m}", name=f"ybn{m}") for m in range(NB)]
            # y_full.T split by MLP pass chunk (so pass A only depends on
            # attention blocks 0..3 under whole-tile dep tracking)
            yfc = [[P0.tile([128, PASSES[p][1]], dt.bfloat16, tag=f"yf{p}_{d}", name=f"yf{p}_{d}")
                    for d in range(ND)] for p in range(3)]
            xs = [[P0.tile([128, N], dt.bfloat16, tag=f"xs{k}_{e}", name=f"xs{k}_{e}")
                   for e in range(ND)] for k in range(2)]
            w2r = [P0.tile([128, D], dt.bfloat16, tag=f"w2r{f}", name=f"w2r{f}") for f in range(NF)]
            b1t = P0.tile([128, NF], dt.float32, tag="b1t", name="b1t")
            b2b = P0.tile([128, D], dt.float32, tag="b2b", name="b2b")
            identb = P0.tile([128, 128], dt.bfloat16, tag="identb", name="identb")
            epsb = P0.tile([128, 1], dt.float32, tag="epsb", name="epsb")
            onep = P0.tile([1, 1], dt.float32, tag="onep", name="onep")
            zerop = P0.tile([128, 1], dt.float32, tag="zerop", name="zerop")

            # ---------------- initial DMAs (ordered by first use) ----------------
            for d in range(ND):
                nc.sync.dma_start(xtb[d][:], xtb_d[128 * d:128 * d + 128, :])
            ap_tiles = []
            cplh = {}
            pet = {}

            def load_cpl(nb):
                t0 = PC.tile([128, 3 * N], dt.bfloat16, tag="cpl", name=f"cplA{nb}")
                t1 = PC.tile([128, 3 * N], dt.bfloat16, tag="cpl", name=f"cplB{nb}")
                nc.sync.dma_start(t0[:], cpl_d[nb, :, 0:3 * N])
                nc.sync.dma_start(t1[:], cpl_d[nb, :, 3 * N:6 * N])
                p = PPET.tile([128, 6], dt.float32, tag="pet", name=f"pet{nb}")
                nc.sync.dma_start(p[:], pem_d[128 * nb:128 * nb + 128, :])
                cplh[nb] = (t0, t1)
                pet[nb] = p

            PAP = tc.alloc_tile_pool(name="pap", bufs=1)
            apre = PAP.tile([128, 2 * ND * D], dt.bfloat16, tag="apre", name="apre")
            nc.sync.dma_start(apre[:, 0:ND * D], ap_d[0])
            nc.sync.dma_start(apre[:, ND * D:2 * ND * D], ap_d[1])
            load_cpl(0)
            for d in range(ND):
                nc.sync.dma_start(ytb[d][:], ytb_d[128 * d:128 * d + 128, :])
            for m in range(NB):
                nc.sync.dma_start(ybn[m][:], ybn_d[128 * m:128 * m + 128, :])
            nc.sync.dma_start(b1t[:], b1t_d[:])
            nc.sync.dma_start(b2b[:], b2b_d[:])
            load_cpl(1)

            # ---------------- init compute ----------------
            make_identity(nc, identb[:])
            nc.gpsimd.memset(epsb[:], 1e-8)
            nc.gpsimd.memset(onep[:], 1.0)
            nc.gpsimd.memset(zerop[:], 0.0)
            nc.sync.dma_start(hmbuf[0:1, 0:1], onep[:])
            nc.sync.dma_start(hmbuf[NP1:NPAD, 0:1], zerop[0:NPAD - NP1, 0:1])
            for d in range(ND):
                # pad cols of the last y_full chunk (col0 = token 1023, rest pad)
                nc.gpsimd.memset(yfc[2][d][:, 1:128], 0.0)
                # CLS column
                nc.scalar.copy(yfc[0][d][:, 0:1], ytb[d][:, 0:1])

            # ---------------- pos chains (issued per block) ----------------
            posg = {}

            def pos_chain(nb):
                """Compute posg(nb) = g * softmax(-pt*pos_logits) rows."""
                c0, c1 = cplh[nb]
                pp = pet[nb]
                tA = WSA.tile([128, N], dt.bfloat16, tag="tA", name=f"tA{nb}")
                nc.vector.tensor_scalar_mul(tA[:], c0[:, 0:N], pp[:, 0:1])
                for c in (1, 2):
                    nc.vector.scalar_tensor_tensor(tA[:], c0[:, c * N:(c + 1) * N],
                                                   pp[:, c:c + 1], tA[:], ALU.mult, ALU.add)
                for c in (3, 4, 5):
                    nc.vector.scalar_tensor_tensor(tA[:], c1[:, (c - 3) * N:(c - 2) * N],
                                                   pp[:, c:c + 1], tA[:], ALU.mult, ALU.add)
                pg = WS.tile([128, N], dt.bfloat16, tag="posg", name=f"posg{nb}")
                zs = SM.tile([128, 1], dt.float32, tag="zs", name=f"zs{nb}")
                nc.scalar.activation(pg[:], tA[:], AF.Exp, scale=-pt, accum_out=zs[:])
                pr = SM.tile([128, 1], dt.float32, tag="pr", name=f"pr{nb}")
                nc.vector.reciprocal(pr[:], zs[:])
                nc.vector.tensor_scalar_mul(pr[:], pr[:], g)
                nc.vector.tensor_scalar_mul(pg[:], pg[:], pr[:])
                posg[nb] = pg

            pos_chain(0)

            # ---------------- prologue: xs GEMMs ----------------
            pos_chain(1)

            # w2r is not needed until the MLP tail; issue it last so the
            # prologue-critical ap/xtb DMAs are not queued behind it.
            for f in range(NF):
                nc.sync.dma_start(w2r[f][:], w2r_d[128 * f:128 * f + 128, :])

            for k in range(2):
                for e in range(ND):
                    for h in range(2):
                        ps = PA.tile([128, 512], dt.float32, tag="psA", name=f"psx{k}_{e}_{h}")
                        for d in range(ND):
                            o = k * ND * D + D * d + 128 * e
                            nc.tensor.matmul(
                                ps[:], apre[:, o:o + 128],
                                xtb[d][:, CLS + 512 * h:CLS + 512 * h + 512],
                                start=(d == 0), stop=(d == ND - 1),
                            )
                        nc.scalar.copy(xs[k][e][:, 512 * h:512 * h + 512], ps[:])
            PAP.release()

            # ---------------- helpers for window stages ----------------
            patches = {}
            ents = {}
            dkas = {}
            acts = {}

            def logits_softmax(nb):
                """logits + softmax exp (no max-sub) + entropy for block nb."""
                r0 = 128 * nb
                pk = []
                ek = []
                es = {}
                # all four softmax exps back-to-back on ACT (one table set)
                for k in range(2):
                    patch = WS.tile([128, N], dt.float32, tag=f"patch{k}", name=f"patch{k}_{nb}")
                    for h in range(2):
                        psl = PA.tile([128, 512], dt.float32, tag="psA", name=f"psl{k}_{h}_{nb}")
                        for e in range(ND):
                            nc.tensor.matmul(
                                psl[:], xs[k][e][:, r0:r0 + 128],
                                ytb[e][:, CLS + 512 * h:CLS + 512 * h + 512],
                                start=(e == 0), stop=(e == ND - 1),
                            )
                        eh = SM.tile([128, 1], dt.float32, tag=f"es{k}{h}", name=f"es{k}{h}_{nb}")
                        nc.scalar.activation(patch[:, 512 * h:512 * h + 512], psl[:],
                                             AF.Exp, scale=scale, accum_out=eh[:])
                        es[(k, h)] = eh
                    pk.append(patch)
                for k in range(2):
                    rk = SM.tile([128, 1], dt.float32, tag=f"rk{k}", name=f"rk{k}_{nb}")
                    nc.vector.tensor_add(rk[:], es[(k, 0)][:], es[(k, 1)][:])
                    nc.vector.reciprocal(rk[:], rk[:])
                    nc.vector.tensor_scalar_mul(rk[:], rk[:], omg)
                    # attn_k = patch*rk + posg  (in place)
                    nc.vector.scalar_tensor_tensor(pk[k][:], pk[k][:], rk[:], posg[nb][:],
                                                   ALU.mult, ALU.add)
                # both lns adjacent on ACT, then entropy reduces on DVE
                lns = []
                for k in range(2):
                    lnk = WSB.tile([128, N], dt.bfloat16, tag=f"lnk{k}", name=f"lnk{k}_{nb}")
                    nc.scalar.activation(lnk[:], pk[k][:], AF.Ln, bias=epsb[:])
                    lns.append(lnk)
                for k in range(2):
                    ent = SM.tile([128, 1], dt.float32, tag=f"ent{k}", name=f"ent{k}_{nb}")
                    nc.vector.tensor_mul(lns[k][:], pk[k][:], lns[k][:])
                    nc.vector.tensor_reduce(ent[:], lns[k][:],
                                            axis=mybir.AxisListType.X, op=ALU.add)
                    ek.append(ent)
                patches[nb] = pk
                ents[nb] = ek

            def routing(nb):
                e0, e1 = ents[nb]
                p0t, p1t = patches[nb]
                rsel = SM.tile([128, 1], dt.float32, tag="rsel", name=f"rsel{nb}")
                nc.vector.tensor_tensor(rsel[:], e0[:], e1[:], ALU.is_ge)
                emax = SM.tile([128, 1], dt.float32, tag="emax", name=f"emax{nb}")
                nc.vector.tensor_tensor(emax[:], e0[:], e1[:], ALU.max)
                vh = SM.tile([128, 1], dt.float32, tag="vh", name=f"vh{nb}")
                nc.scalar.activation(vh[:], emax[:], AF.Exp, scale=-ht)
                nc.vector.tensor_scalar_add(vh[:], vh[:], 1.0)
                nc.vector.reciprocal(vh[:], vh[:])
                ha = SM.tile([128, 1], dt.float32, tag="ha", name=f"ha{nb}")
                nc.vector.tensor_scalar_mul(ha[:], vh[:], 2.0)
                nc.sync.dma_start(hmbuf[CLS + 128 * nb:CLS + 128 * nb + 128, 0:1], ha[:])
                # dka = attn1 + rsel*(attn0-attn1)
                nc.vector.tensor_sub(p0t[:], p0t[:], p1t[:])
                dk = WSD.tile([128, N], dt.bfloat16, tag="dka", name=f"dka{nb}")
                nc.vector.scalar_tensor_tensor(dk[:], p0t[:], rsel[:], p1t[:],
                                               ALU.mult, ALU.add)
                dkas[nb] = dk

            def transp(nb):
                dk = dkas[nb]
                pst = PT.tile([128, N], dt.bfloat16, tag="psT", name=f"pst{nb}")
                for mb in range(NB):
                    nc.tensor.transpose(pst[:, 128 * mb:128 * mb + 128],
                                        dk[:, 128 * mb:128 * mb + 128], identb[:])
                at = []
                for mb in range(NB):
                    a = PACT.tile([128, 128], dt.bfloat16, tag=f"acT{mb}", name=f"acT{mb}_{nb}")
                    nc.scalar.copy(a[:], pst[:, 128 * mb:128 * mb + 128])
                    at.append(a)
                acts[nb] = at

            def yout(nb):
                """y_outT columns for attention block nb -> yfc chunk tiles."""
                at = acts[nb]
                c0 = CLS + 128 * nb          # global yfT col range [c0, c0+128)
                for d in range(ND):
                    psy = PH.tile([128, 512], dt.float32, tag="psH", name=f"psy{d}_{nb}")
                    for mb in range(NB):
                        nc.tensor.matmul(
                            psy[:, 0:128], ybn[mb][:, 128 * d:128 * d + 128], at[mb][:],
                            start=(mb == 0), stop=(mb == NB - 1),
                        )
                    # scatter into pass-chunk tiles (may straddle two chunks)
                    c = c0
                    off = 0
                    while off < 128:
                        p = c // 512 if c < 1024 else 2
                        ps0, pw = PASSES[p]
                        w = min(128 - off, ps0 + pw - c)
                        nc.scalar.copy(yfc[p][d][:, c - ps0:c - ps0 + w],
                                       psy[:, off:off + w])
                        c += w
                        off += w

            def mlp1_f(p, f, wtile):
                """One f-row of MLP1 for pass p -> gel tile."""
                s0, wd = PASSES[p]
                psh = PH.tile([128, 512], dt.float32, tag="psH", name=f"psh{p}_{f}")
                for c in range(NC2):
                    rhs = xtb[c][:, s0:s0 + wd] if c < ND else yfc[p][c - ND][:, 0:wd]
                    nc.tensor.matmul(psh[:, 0:wd], wtile[:, 128 * c:128 * c + 128], rhs,
                                     start=(c == 0), stop=(c == NC2 - 1))
                gl = PGEL.tile([128, 512], dt.bfloat16, tag=f"gel{f}", name=f"gel{p}_{f}")
                nc.scalar.activation(gl[:, 0:wd], psh[:, 0:wd], AF.Gelu, bias=b1t[:, f:f + 1])
                return gl

            gel_cur = [None] * NF

            def mlp2_block(p, tb):
                """MLP2 + residual + heat + store for output token block tb."""
                s0, _ = PASSES[p]
                rel = 128 * tb - s0
                r0 = 128 * tb
                nrows = min(128, NP1 - r0)
                hmc = SM.tile([128, 1], dt.float32, tag="hmc", name=f"hmc{tb}")
                nc.sync.dma_start(hmc[:], hmbuf[r0:r0 + 128, 0:1])
                for ch in range(3):
                    cs = 256 * ch
                    xn = PXNA.tile([128, 256], dt.float32, tag="xna", name=f"xna{tb}_{ch}")
                    nc.sync.dma_start(xn[:], xna_d[r0:r0 + 128, cs:cs + 256])
                    pso = PO.tile([128, 256], dt.float32, tag="psO", name=f"pso{tb}_{ch}")
                    for f in range(NF):
                        nc.tensor.matmul(pso[:], gel_cur[f][:, rel:rel + 128],
                                         w2r[f][:, cs:cs + 256],
                                         start=(f == 0), stop=(f == NF - 1))
                    st = POT.tile([128, 256], dt.float32, tag="st", name=f"st{tb}_{ch}")
                    nc.vector.tensor_add(st[:], pso[:], b2b[:, cs:cs + 256])
                    ot = POT.tile([128, 256], dt.float32, tag="ot", name=f"ot{tb}_{ch}")
                    nc.vector.scalar_tensor_tensor(ot[:], st[:], hmc[:],
                                                   xn[:], ALU.mult, ALU.add)
                    nc.sync.dma_start(out_d[r0:r0 + nrows, cs:cs + 256], ot[:nrows, :])

            # ---------------- windows + MLP (phase-2 pools reuse PAP space) --
            with tc.tile_pool(name="pgel", bufs=1) as PGEL, \
                 tc.tile_pool(name="pw1", bufs=3) as PW1, \
                 tc.tile_pool(name="pxna", bufs=2) as PXNA, \
                 tc.tile_pool(name="pot", bufs=2) as POT:
                for nb in range(NB):
                    if nb + 2 < NB:
                        load_cpl(nb + 2)
                    w1_tiles = []
                    if nb >= 5:
                        # stream W1 rows for this window's pass-A slice
                        for f in range(8 * (nb - 5), 8 * (nb - 5) + 8):
                            w = PW1.tile([128, NC2 * 128], dt.bfloat16, tag="w1", name=f"w1A{f}")
                            nc.sync.dma_start(w[:], w1p_d[f])
                            w1_tiles.append((f, w))
                    logits_softmax(nb)
                    if nb + 1 < NB:
                        pos_chain(nb + 1)
                    routing(nb)
                    if nb >= 2:
                        transp(nb - 2)
                        yout(nb - 2)
                    for f, w in w1_tiles:
                        gel_cur[f] = mlp1_f(0, f, w)

                for nb in (NB - 2, NB - 1):
                    transp(nb)
                    yout(nb)

                # ---------------- MLP tail ----------------
                for tb in range(4):
                    mlp2_block(0, tb)
                for p in (1, 2):
                    for f in range(NF):
                        w = PW1.tile([128, NC2 * 128], dt.bfloat16, tag="w1", name=f"w1{p}_{f}")
                        nc.sync.dma_start(w[:], w1p_d[f])
                        gel_cur[f] = mlp1_f(p, f, w)
                    for tb in range(4 * p, min(4 * p + 4, NTB)):
                        mlp2_block(p, tb)

    nc.compile()
    return nc


def _get_prog(g, ht, pt):
    key = (round(float(g), 9), round(float(ht), 9), round(float(pt), 9))
    if key not in _prog_cache:
        _prog_cache[key] = _build(*key)
    return _prog_cache[key]


def kernel(x, y, coords, U, S1, S2, gating, h_temp, p_temp, pos_emb, W1, b1, W2, b2):
    x = np.asarray(x, dtype=np.float32)
    y = np.asarray(y, dtype=np.float32)
    coords = np.asarray(coords, dtype=np.float32)
    U = np.asarray(U, dtype=np.float32)
    bf16 = ml_dtypes.bfloat16

    g = float(1.0 / (1.0 + np.exp(-float(np.asarray(gating)))))
    ht = float(np.asarray(h_temp))
    pt = abs(float(np.asarray(p_temp)))
    assert ht > 0.0
    nc = _get_prog(g, ht, pt)

    # ---- shared (replicated) host prep ----
    app = np.empty((2, 128, ND * D), bf16)
    for k, S in enumerate((S1, S2)):
        A = (U.T * np.abs(np.asarray(S, np.float32))[None, :]) @ U
        app[k] = A.reshape(ND, 128, D).transpose(1, 0, 2).reshape(128, ND * D).astype(bf16)
    cplp = np.ascontiguousarray(
        coords.reshape(NB, 128, N, 6).transpose(0, 1, 3, 2).reshape(NB, 128, 6 * N)
    ).astype(bf16)
    pem = np.ascontiguousarray(np.asarray(pos_emb, np.float32)[:, :, 0])
    W1f = np.asarray(W1, np.float32)
    w1p = np.ascontiguousarray(
        W1f.reshape(NC2, 128, NF, 128).transpose(2, 1, 0, 3).reshape(NF, 128, NC2 * 128)
    ).astype(bf16)
    w2r = np.asarray(W2, np.float32).astype(bf16)
    b1t = np.ascontiguousarray(np.asarray(b1, np.float32).reshape(NF, 128).T)
    b2b = np.broadcast_to(np.asarray(b2, np.float32), (128, D)).copy()
    shared = {"app": app, "cplp": cplp, "pem": pem, "w1p": w1p, "w2r": w2r,
              "b1t": b1t, "b2b": b2b}

    in_maps = []
    for b in range(B):
        xtbp = np.zeros((D, NPAD), bf16)
        xtbp[:, :NP1] = x[b].T.astype(bf16)
        xna = np.zeros((NPAD, D), np.float32)
        xna[:NP1] = x[b]
        m = dict(shared)
        m["xtb"] = xtbp
        m["ytb"] = np.ascontiguousarray(y[b].T).astype(bf16)
        m["ybn"] = y[b, CLS:, :].astype(bf16)
        m["xna"] = xna
        in_maps.append(m)

    res = run_bass_kernel_spmd(nc, in_maps, list(range(B)))
    out = np.stack([res.results[b]["out"][:NP1, :] for b in range(B)])
    return out.astype(np.float32)


if __name__ == "__main__":
    import time
    sys.path.insert(0, "/root/problem")
    from reference import setup_inputs, reference

    inp = {k: np.asarray(v) for k, v in setup_inputs().items()}
    t0 = time.time()
    got = kernel(**inp)
    print("kernel wall:", time.time() - t0)
    exp = np.asarray(reference(**inp))
    d = np.abs(got - exp)
    print("absmax_rel:", d.max() / np.abs(exp).max())
    print("rms_rel:", np.sqrt((d ** 2).mean()) / np.sqrt((exp ** 2).mean()))


# revision 33
# speedup vs baseline: 1.0090x; 1.0090x over previous
"""Trainium2 Bass kernel for nn_DepthCueRectification_Sp.

Data-parallel over batch: 8 batch elements -> 8 NeuronCores (SPMD).

Fully software-pipelined single-pass design (all GEMMs bf16):
  prologue: xs_k.T = A_k.T @ xb.T   (A_k = U.T diag|S_k| U folded on host)
            pos chains for blocks 0,1 on DVE/Pool
  windows nb=0..7 (one attention row-block each):
    tensor: logits(nb) -> transpose(nb-1) -> y_outT(nb-1) -> MLP1 pass-A slice
    ACT:    softmax exps (no max-sub), entropy ln, psum drains, gelu
    DVE:    pos chain A, entropy fused mul+reduce, routing
    Pool:   pos chain B, attn mix, route-combine (dka)
  tail: heat shift-matmul, MLP2-A, MLP1-B, MLP2-B, MLP1-C, MLP2-C,
        residual + heat scaling + store per 256-col chunk.

Routing simplification: hmap0>=hmap1 <=> entS0>=entS1 where
entS = sum(attn*ln(attn+eps)) (= -entropy), and
heat = 2/(1+exp(-h_temp*max(entS0,entS1))). The CLS row's heat=1 and the
(token -> output row+1) shift are realized with a tiny shift-matrix matmul.
"""

import sys

if "/opt/trn_rl_repo" not in sys.path:
    sys.path.insert(0, "/opt/trn_rl_repo")

import numpy as np
import ml_dtypes

import concourse.bass as bass
import concourse.mybir as mybir
import concourse.tile as tile
from concourse import bacc
from concourse.bass_utils import run_bass_kernel_spmd
from concourse.masks import make_identity

B, N, D, DFF, CLS = 8, 1024, 768, 3072, 1
NP1 = N + CLS          # 1025
NPAD = 1152            # 9 * 128
ND = D // 128          # 6
NB = N // 128          # 8
NF = DFF // 128        # 24
NC2 = 2 * D // 128     # 12
NTB = NPAD // 128      # 9 output token blocks
AF = mybir.ActivationFunctionType
ALU = mybir.AluOpType
dt = mybir.dt

# MLP token-column passes over the padded 1152 columns
PASSES = [(0, 512), (512, 512), (1024, 128)]

_prog_cache = {}


def _build(g, ht, pt):
    scale = float(D) ** -0.5
    omg = 1.0 - g

    nc = bacc.Bacc("TRN2", target_bir_lowering=False, debug=False, num_devices=8)

    xtb_d = nc.declare_dram_parameter("xtb", [D, NPAD], dt.bfloat16, isOutput=False)
    ytb_d = nc.declare_dram_parameter("ytb", [D, NP1], dt.bfloat16, isOutput=False)
    ybn_d = nc.declare_dram_parameter("ybn", [N, D], dt.bfloat16, isOutput=False)
    ap_d = nc.declare_dram_parameter("app", [2, 128, ND * D], dt.bfloat16, isOutput=False)
    cpl_d = nc.declare_dram_parameter("cplp", [NB, 128, 6 * N], dt.bfloat16, isOutput=False)
    pem_d = nc.declare_dram_parameter("pem", [N, 6], dt.float32, isOutput=False)
    w1p_d = nc.declare_dram_parameter("w1p", [NF, 128, NC2 * 128], dt.bfloat16, isOutput=False)
    w2r_d = nc.declare_dram_parameter("w2r", [DFF, D], dt.bfloat16, isOutput=False)
    b1t_d = nc.declare_dram_parameter("b1t", [128, NF], dt.float32, isOutput=False)
    b2b_d = nc.declare_dram_parameter("b2b", [128, D], dt.float32, isOutput=False)
    xna_d = nc.declare_dram_parameter("xna", [NPAD, D], dt.float32, isOutput=False)
    out_d = nc.declare_dram_parameter("out", [NPAD, D], dt.float32, isOutput=True)
    hmbuf = nc.dram_tensor("hmbuf", [NPAD, 1], dt.float32)

    with tile.TileContext(nc) as tc:
        with tc.tile_pool(name="p0", bufs=1) as P0, \
             tc.tile_pool(name="pws", bufs=2) as WS, \
             tc.tile_pool(name="pwA", bufs=1) as WSA, \
             tc.tile_pool(name="pwB", bufs=1) as WSB, \
             tc.tile_pool(name="pcpl", bufs=2) as PC, \
             tc.tile_pool(name="ppet", bufs=3) as PPET, \
             tc.tile_pool(name="psml", bufs=2) as SM, \
             tc.tile_pool(name="pact", bufs=1) as PACT, \
             tc.tile_pool(name="psA", bufs=3, space="PSUM") as PA, \
             tc.tile_pool(name="psT", bufs=1, space="PSUM") as PT, \
             tc.tile_pool(name="psH", bufs=2, space="PSUM") as PH, \
             tc.tile_pool(name="psO", bufs=2, space="PSUM") as PO:

            # ---------------- persistent tiles ----------------
            xtb = [P0.tile([128, NPAD], dt.bfloat16, tag=f"xtb{d}", name=f"xtb{d}") for d in range(ND)]
            ytb = [P0.tile([128, NP1], dt.bfloat16, tag=f"ytb{d}", name=f"ytb{d}") for d in range(ND)]
            ybn = [P0.tile([128, D], dt.bfloat16, tag=f"ybn{m}", name=f"ybn{m}") for m in range(NB)]
            # y_full.T split by MLP pass chunk (so pass A only depends on
            # attention blocks 0..3 under whole-tile dep tracking)
            yfc = [[P0.tile([128, PASSES[p][1]], dt.bfloat16, tag=f"yf{p}_{d}", name=f"yf{p}_{d}")
                    for d in range(ND)] for p in range(3)]
            xs = [[P0.tile([128, N], dt.bfloat16, tag=f"xs{k}_{e}", name=f"xs{k}_{e}")
                   for e in range(ND)] for k in range(2)]
            w2r = [P0.tile([128, D], dt.bfloat16, tag=f"w2r{f}", name=f"w2r{f}") for f in range(NF)]
            b1t = P0.tile([128, NF], dt.float32, tag="b1t", name="b1t")
            b2b = P0.tile([128, D], dt.float32, tag="b2b", name="b2b")
            identb = P0.tile([128, 128], dt.bfloat16, tag="identb", name="identb")
            epsb = P0.tile([128, 1], dt.float32, tag="epsb", name="epsb")
            onep = P0.tile([1, 1], dt.float32, tag="onep", name="onep")
            zerop = P0.tile([128, 1], dt.float32, tag="zerop", name="zerop")

            # ---------------- initial DMAs (ordered by first use) ----------------
            for d in range(ND):
                nc.sync.dma_start(xtb[d][:], xtb_d[128 * d:128 * d + 128, :])
            ap_tiles = []
            cplh = {}
            pet = {}

            def load_cpl(nb):
                t0 = PC.tile([128, 3 * N], dt.bfloat16, tag="cpl", name=f"cplA{nb}")
                t1 = PC.tile([128, 3 * N], dt.bfloat16, tag="cpl", name=f"cplB{nb}")
                nc.sync.dma_start(t0[:], cpl_d[nb, :, 0:3 * N])
                nc.sync.dma_start(t1[:], cpl_d[nb, :, 3 * N:6 * N])
                p = PPET.tile([128, 6], dt.float32, tag="pet", name=f"pet{nb}")
                nc.sync.dma_start(p[:], pem_d[128 * nb:128 * nb + 128, :])
                cplh[nb] = (t0, t1)
                pet[nb] = p

            PAP = tc.alloc_tile_pool(name="pap", bufs=1)
            apre = PAP.tile([128, 2 * ND * D], dt.bfloat16, tag="apre", name="apre")
            nc.sync.dma_start(apre[:, 0:ND * D], ap_d[0])
            nc.sync.dma_start(apre[:, ND * D:2 * ND * D], ap_d[1])
            load_cpl(0)
            for d in range(ND):
                nc.sync.dma_start(ytb[d][:], ytb_d[128 * d:128 * d + 128, :])
            for m in range(NB):
                nc.sync.dma_start(ybn[m][:], ybn_d[128 * m:128 * m + 128, :])
            nc.sync.dma_start(b1t[:], b1t_d[:])
            nc.sync.dma_start(b2b[:], b2b_d[:])
            load_cpl(1)

            # ---------------- init compute ----------------
            make_identity(nc, identb[:])
            nc.gpsimd.memset(epsb[:], 1e-8)
            nc.gpsimd.memset(onep[:], 1.0)
            nc.gpsimd.memset(zerop[:], 0.0)
            nc.sync.dma_start(hmbuf[0:1, 0:1], onep[:])
            nc.sync.dma_start(hmbuf[NP1:NPAD, 0:1], zerop[0:NPAD - NP1, 0:1])
            for d in range(ND):
                # pad cols of the last y_full chunk (col0 = token 1023, rest pad)
                nc.gpsimd.memset(yfc[2][d][:, 1:128], 0.0)
                # CLS column
                nc.scalar.copy(yfc[0][d][:, 0:1], ytb[d][:, 0:1])

            # ---------------- pos chains (issued per block) ----------------
            posg = {}

            def pos_chain(nb):
                """Compute posg(nb) = g * softmax(-pt*pos_logits) rows."""
                c0, c1 = cplh[nb]
                pp = pet[nb]
                tA = WSA.tile([128, N], dt.bfloat16, tag="tA", name=f"tA{nb}")
                nc.vector.tensor_scalar_mul(tA[:], c0[:, 0:N], pp[:, 0:1])
                for c in (1, 2):
                    nc.vector.scalar_tensor_tensor(tA[:], c0[:, c * N:(c + 1) * N],
                                                   pp[:, c:c + 1], tA[:], ALU.mult, ALU.add)
                for c in (3, 4, 5):
                    nc.vector.scalar_tensor_tensor(tA[:], c1[:, (c - 3) * N:(c - 2) * N],
                                                   pp[:, c:c + 1], tA[:], ALU.mult, ALU.add)
                pg = WS.tile([128, N], dt.bfloat16, tag="posg", name=f"posg{nb}")
                zs = SM.tile([128, 1], dt.float32, tag="zs", name=f"zs{nb}")
                nc.scalar.activation(pg[:], tA[:], AF.Exp, scale=-pt, accum_out=zs[:])
                pr = SM.tile([128, 1], dt.float32, tag="pr", name=f"pr{nb}")
                nc.vector.reciprocal(pr[:], zs[:])
                nc.vector.tensor_scalar_mul(pr[:], pr[:], g)
                nc.vector.tensor_scalar_mul(pg[:], pg[:], pr[:])
                posg[nb] = pg

            pos_chain(0)

            # ---------------- prologue: xs GEMMs ----------------
            pos_chain(1)

            for k in range(2):
                for e in range(ND):
                    for h in range(2):
                        ps = PA.tile([128, 512], dt.float32, tag="psA", name=f"psx{k}_{e}_{h}")
                        for d in range(ND):
                            o = k * ND * D + D * d + 128 * e
                            nc.tensor.matmul(
                                ps[:], apre[:, o:o + 128],
                                xtb[d][:, CLS + 512 * h:CLS + 512 * h + 512],
                                start=(d == 0), stop=(d == ND - 1),
                            )
                        nc.scalar.copy(xs[k][e][:, 512 * h:512 * h + 512], ps[:])
            PAP.release()

            # ---------------- helpers for window stages ----------------
            patches = {}
            ents = {}
            dkas = {}
            acts = {}

            def logits_softmax(nb):
                """logits + softmax exp (no max-sub) + entropy for block nb."""
                r0 = 128 * nb
                pk = []
                ek = []
                es = {}
                # all four softmax exps back-to-back on ACT (one table set)
                for k in range(2):
                    patch = WS.tile([128, N], dt.float32, tag=f"patch{k}", name=f"patch{k}_{nb}")
                    for h in range(2):
                        psl = PA.tile([128, 512], dt.float32, tag="psA", name=f"psl{k}_{h}_{nb}")
                        for e in range(ND):
                            nc.tensor.matmul(
                                psl[:], xs[k][e][:, r0:r0 + 128],
                                ytb[e][:, CLS + 512 * h:CLS + 512 * h + 512],
                                start=(e == 0), stop=(e == ND - 1),
                            )
                        eh = SM.tile([128, 1], dt.float32, tag=f"es{k}{h}", name=f"es{k}{h}_{nb}")
                        nc.scalar.activation(patch[:, 512 * h:512 * h + 512], psl[:],
                                             AF.Exp, scale=scale, accum_out=eh[:])
                        es[(k, h)] = eh
                    pk.append(patch)
                for k in range(2):
                    rk = SM.tile([128, 1], dt.float32, tag=f"rk{k}", name=f"rk{k}_{nb}")
                    nc.vector.tensor_add(rk[:], es[(k, 0)][:], es[(k, 1)][:])
                    nc.vector.reciprocal(rk[:], rk[:])
                    nc.vector.tensor_scalar_mul(rk[:], rk[:], omg)
                    # attn_k = patch*rk + posg  (in place)
                    nc.vector.scalar_tensor_tensor(pk[k][:], pk[k][:], rk[:], posg[nb][:],
                                                   ALU.mult, ALU.add)
                # both lns adjacent on ACT, then entropy reduces on DVE
                lns = []
                for k in range(2):
                    lnk = WSB.tile([128, N], dt.float32, tag=f"lnk{k}", name=f"lnk{k}_{nb}")
                    nc.scalar.activation(lnk[:], pk[k][:], AF.Ln, bias=epsb[:])
                    lns.append(lnk)
                for k in range(2):
                    ent = SM.tile([128, 1], dt.float32, tag=f"ent{k}", name=f"ent{k}_{nb}")
                    nc.vector.tensor_mul(lns[k][:], pk[k][:], lns[k][:])
                    nc.vector.tensor_reduce(ent[:], lns[k][:],
                                            axis=mybir.AxisListType.X, op=ALU.add)
                    ek.append(ent)
                patches[nb] = pk
                ents[nb] = ek

            def routing(nb):
                e0, e1 = ents[nb]
                p0t, p1t = patches[nb]
                rsel = SM.tile([128, 1], dt.float32, tag="rsel", name=f"rsel{nb}")
                nc.vector.tensor_tensor(rsel[:], e0[:], e1[:], ALU.is_ge)
                emax = SM.tile([128, 1], dt.float32, tag="emax", name=f"emax{nb}")
                nc.vector.tensor_tensor(emax[:], e0[:], e1[:], ALU.max)
                vh = SM.tile([128, 1], dt.float32, tag="vh", name=f"vh{nb}")
                nc.scalar.activation(vh[:], emax[:], AF.Exp, scale=-ht)
                nc.vector.tensor_scalar_add(vh[:], vh[:], 1.0)
                nc.vector.reciprocal(vh[:], vh[:])
                ha = SM.tile([128, 1], dt.float32, tag="ha", name=f"ha{nb}")
                nc.vector.tensor_scalar_mul(ha[:], vh[:], 2.0)
                nc.sync.dma_start(hmbuf[CLS + 128 * nb:CLS + 128 * nb + 128, 0:1], ha[:])
                # dka = attn1 + rsel*(attn0-attn1)
                nc.vector.tensor_sub(p0t[:], p0t[:], p1t[:])
                dk = WS.tile([128, N], dt.bfloat16, tag="dka", name=f"dka{nb}")
                nc.vector.scalar_tensor_tensor(dk[:], p0t[:], rsel[:], p1t[:],
                                               ALU.mult, ALU.add)
                dkas[nb] = dk

            def transp(nb):
                dk = dkas[nb]
                pst = PT.tile([128, N], dt.bfloat16, tag="psT", name=f"pst{nb}")
                for mb in range(NB):
                    nc.tensor.transpose(pst[:, 128 * mb:128 * mb + 128],
                                        dk[:, 128 * mb:128 * mb + 128], identb[:])
                at = []
                for mb in range(NB):
                    a = PACT.tile([128, 128], dt.bfloat16, tag=f"acT{mb}", name=f"acT{mb}_{nb}")
                    nc.scalar.copy(a[:], pst[:, 128 * mb:128 * mb + 128])
                    at.append(a)
                acts[nb] = at

            def yout(nb):
                """y_outT columns for attention block nb -> yfc chunk tiles."""
                at = acts[nb]
                c0 = CLS + 128 * nb          # global yfT col range [c0, c0+128)
                for d in range(ND):
                    psy = PH.tile([128, 512], dt.float32, tag="psH", name=f"psy{d}_{nb}")
                    for mb in range(NB):
                        nc.tensor.matmul(
                            psy[:, 0:128], ybn[mb][:, 128 * d:128 * d + 128], at[mb][:],
                            start=(mb == 0), stop=(mb == NB - 1),
                        )
                    # scatter into pass-chunk tiles (may straddle two chunks)
                    c = c0
                    off = 0
                    while off < 128:
                        p = c // 512 if c < 1024 else 2
                        ps0, pw = PASSES[p]
                        w = min(128 - off, ps0 + pw - c)
                        nc.scalar.copy(yfc[p][d][:, c - ps0:c - ps0 + w],
                                       psy[:, off:off + w])
                        c += w
                        off += w

            def mlp1_f(p, f, wtile):
                """One f-row of MLP1 for pass p -> gel tile."""
                s0, wd = PASSES[p]
                psh = PH.tile([128, 512], dt.float32, tag="psH", name=f"psh{p}_{f}")
                for c in range(NC2):
                    rhs = xtb[c][:, s0:s0 + wd] if c < ND else yfc[p][c - ND][:, 0:wd]
                    nc.tensor.matmul(psh[:, 0:wd], wtile[:, 128 * c:128 * c + 128], rhs,
                                     start=(c == 0), stop=(c == NC2 - 1))
                gl = PGEL.tile([128, 512], dt.bfloat16, tag=f"gel{f}", name=f"gel{p}_{f}")
                nc.scalar.activation(gl[:, 0:wd], psh[:, 0:wd], AF.Gelu, bias=b1t[:, f:f + 1])
                return gl

            gel_cur = [None] * NF

            def mlp2_block(p, tb):
                """MLP2 + residual + heat + store for output token block tb."""
                s0, _ = PASSES[p]
                rel = 128 * tb - s0
                r0 = 128 * tb
                nrows = min(128, NP1 - r0)
                hmc = SM.tile([128, 1], dt.float32, tag="hmc", name=f"hmc{tb}")
                nc.sync.dma_start(hmc[:], hmbuf[r0:r0 + 128, 0:1])
                for ch in range(3):
                    cs = 256 * ch
                    xn = PXNA.tile([128, 256], dt.float32, tag="xna", name=f"xna{tb}_{ch}")
                    nc.sync.dma_start(xn[:], xna_d[r0:r0 + 128, cs:cs + 256])
                    pso = PO.tile([128, 256], dt.float32, tag="psO", name=f"pso{tb}_{ch}")
                    for f in range(NF):
                        nc.tensor.matmul(pso[:], gel_cur[f][:, rel:rel + 128],
                                         w2r[f][:, cs:cs + 256],
                                         start=(f == 0), stop=(f == NF - 1))
                    st = POT.tile([128, 256], dt.float32, tag="st", name=f"st{tb}_{ch}")
                    nc.vector.tensor_add(st[:], pso[:], b2b[:, cs:cs + 256])
                    ot = POT.tile([128, 256], dt.float32, tag="ot", name=f"ot{tb}_{ch}")
                    nc.vector.scalar_tensor_tensor(ot[:], st[:], hmc[:],
                                                   xn[:], ALU.mult, ALU.add)
                    nc.sync.dma_start(out_d[r0:r0 + nrows, cs:cs + 256], ot[:nrows, :])

            # ---------------- windows + MLP (phase-2 pools reuse PAP space) --
            with tc.tile_pool(name="pgel", bufs=1) as PGEL, \
                 tc.tile_pool(name="pw1", bufs=3) as PW1, \
                 tc.tile_pool(name="pxna", bufs=2) as PXNA, \
                 tc.tile_pool(name="pot", bufs=2) as POT:
                for nb in range(NB):
                    if nb + 2 < NB:
                        load_cpl(nb + 2)
                    w1_tiles = []
                    if nb >= 4:
                        # stream W1 rows for this window's pass-A slice
                        for f in range(6 * (nb - 4), 6 * (nb - 4) + 6):
                            w = PW1.tile([128, NC2 * 128], dt.bfloat16, tag="w1", name=f"w1A{f}")
                            nc.sync.dma_start(w[:], w1p_d[f])
                            w1_tiles.append((f, w))
                        # w2r (needed from MLP2-A) trickles in behind the w1
                        # stream instead of ahead of the cpl loads
                        for f in range(6 * (nb - 4), 6 * (nb - 4) + 6):
                            nc.sync.dma_start(w2r[f][:], w2r_d[128 * f:128 * f + 128, :])
                    logits_softmax(nb)
                    if nb + 1 < NB:
                        pos_chain(nb + 1)
                    routing(nb)
                    if nb >= 1:
                        transp(nb - 1)
                        yout(nb - 1)
                    for f, w in w1_tiles:
                        gel_cur[f] = mlp1_f(0, f, w)

                transp(NB - 1)
                yout(NB - 1)

                # ---------------- MLP tail ----------------
                for tb in range(4):
                    mlp2_block(0, tb)
                for p in (1, 2):
                    for f in range(NF):
                        w = PW1.tile([128, NC2 * 128], dt.bfloat16, tag="w1", name=f"w1{p}_{f}")
                        nc.sync.dma_start(w[:], w1p_d[f])
                        gel_cur[f] = mlp1_f(p, f, w)
                    for tb in range(4 * p, min(4 * p + 4, NTB)):
                        mlp2_block(p, tb)

    nc.compile()
    return nc


def _get_prog(g, ht, pt):
    key = (round(float(g), 9), round(float(ht), 9), round(float(pt), 9))
    if key not in _prog_cache:
        _prog_cache[key] = _build(*key)
    return _prog_cache[key]


def kernel(x, y, coords, U, S1, S2, gating, h_temp, p_temp, pos_emb, W1, b1, W2, b2):
    x = np.asarray(x, dtype=np.float32)
    y = np.asarray(y, dtype=np.float32)
    coords = np.asarray(coords, dtype=np.float32)
    U = np.asarray(U, dtype=np.float32)
    bf16 = ml_dtypes.bfloat16

    g = float(1.0 / (1.0 + np.exp(-float(np.asarray(gating)))))
    ht = float(np.asarray(h_temp))
    pt = abs(float(np.asarray(p_temp)))
    assert ht > 0.0
    nc = _get_prog(g, ht, pt)

    # ---- shared (replicated) host prep ----
    app = np.empty((2, 128, ND * D), bf16)
    for k, S in enumerate((S1, S2)):
        A = (U.T * np.abs(np.asarray(S, np.float32))[None, :]) @ U
        app[k] = A.reshape(ND, 128, D).transpose(1, 0, 2).reshape(128, ND * D).astype(bf16)
    cplp = np.ascontiguousarray(
        coords.reshape(NB, 128, N, 6).transpose(0, 1, 3, 2).reshape(NB, 128, 6 * N)
    ).astype(bf16)
    pem = np.ascontiguousarray(np.asarray(pos_emb, np.float32)[:, :, 0])
    W1f = np.asarray(W1, np.float32)
    w1p = np.ascontiguousarray(
        W1f.reshape(NC2, 128, NF, 128).transpose(2, 1, 0, 3).reshape(NF, 128, NC2 * 128)
    ).astype(bf16)
    w2r = np.asarray(W2, np.float32).astype(bf16)
    b1t = np.ascontiguousarray(np.asarray(b1, np.float32).reshape(NF, 128).T)
    b2b = np.broadcast_to(np.asarray(b2, np.float32), (128, D)).copy()
    shared = {"app": app, "cplp": cplp, "pem": pem, "w1p": w1p, "w2r": w2r,
              "b1t": b1t, "b2b": b2b}

    in_maps = []
    for b in range(B):
        xtbp = np.zeros((D, NPAD), bf16)
        xtbp[:, :NP1] = x[b].T.astype(bf16)
        xna = np.zeros((NPAD, D), np.float32)
        xna[:NP1] = x[b]
        m = dict(shared)
        m["xtb"] = xtbp
        m["ytb"] = np.ascontiguousarray(y[b].T).astype(bf16)
        m["ybn"] = y[b, CLS:, :].astype(bf16)
        m["xna"] = xna
        in_maps.append(m)

    res = run_bass_kernel_spmd(nc, in_maps, list(range(B)))
    out = np.stack([res.results[b]["out"][:NP1, :] for b in range(B)])
    return out.astype(np.float32)


if __name__ == "__main__":
    import time
    sys.path.insert(0, "/root/problem")
    from reference import setup_inputs, reference

    inp = {k: np.asarray(v) for k, v in setup_inputs().items()}
    t0 = time.time()
    got = kernel(**inp)
    print("kernel wall:", time.time() - t0)
    exp = np.asarray(reference(**inp))
    d = np.abs(got - exp)
    print("absmax_rel:", d.max() / np.abs(exp).max())
    print("rms_rel:", np.sqrt((d ** 2).mean()) / np.sqrt((exp ** 2).mean()))
